# revision 5
# baseline (speedup 1.0000x reference)
"""KAN layer as a fused fp16 matmul kernel on 8 TRN2 cores (v4).

Same math as v2/v3 (flip-stabilized truncated powers, fp16 operands,
fp32 PSUM accumulation) with a latency-optimized schedule:
- DMA issues all on the Sync queue, x first (spreading them across
  engines was measured slower: transfers landed later)
- identity matrix rides in the fp16 weight blob (bitcast to f32 on chip)
- xT->fp16 cast copies split across Scalar and Vector (the copy IS the
  x feature plane); one act-table load (first scalar op is Silu)
- engine split from trace data: Vector 6 fused relu^3 planes, Scalar
  silu/square + the relu/square pair of the tiniest plane (z4), GpSimd
  x^3 and the z4 multiply; output copies split Scalar/Vector.
"""

import os
import threading

import numpy as np

IN = 256
OUT = 256
BATCH = 2048
N_CORES = 8
B_SHARD = BATCH // N_CORES          # 256
K = 3
NUM = 8
H = 2.0 / NUM
G = NUM + 1 + 2 * K
N_COEF = NUM + K
KNOTS = -1.0 - K * H + H * np.arange(G)      # t_j = -1.75 + 0.25 j
KAPPA = 1.0 / (6.0 * H ** 3)
BINOM = (1.0, -4.0, 6.0, -4.0, 1.0)

J_ALL = (4, 5, 6, 7, 8, 9, 10)
J_FLIP = (4, 5, 6)                   # t<0: feature relu(t-x)^3
# chunk consumption order, by expected feature readiness
PLANE_ORDER = ["ones", "x", "x2", "sil", "z8", "z9", "z10", "x3",
               "z4", "z5", "z6", "z7"]
N_CHUNKS = 1 + 2 * (len(PLANE_ORDER) - 1)    # 23
W_COLS = N_CHUNKS * 256
# weight DMA group boundaries (chunk indices)
W_GROUPS = (0, 3, 9, 16, N_CHUNKS)
J_SHIFT = (7, 8, 9, 10)              # t>=0: relu(x-t)^3
VEC_PLANES = ("z8", "z9", "z10", "z5")               # vector, both halves
SPLIT_PLANES = ("z6", "z7")          # h0 on vector, h1 via scalar+gpsimd
SG_PLANES = ("z4",)                  # scalar relu+sq (f16) + gpsimd mul


def _build_tables(control_points, scale_base, scale_spline, mask):
    cp = np.asarray(control_points, np.float64)
    ss = np.asarray(mask, np.float64) * np.asarray(scale_spline, np.float64)
    sb = np.asarray(mask, np.float64) * np.asarray(scale_base, np.float64)
    Wx = [np.zeros((IN, OUT)) for _ in range(4)]
    Wr = {j: np.zeros((IN, OUT)) for j in J_ALL}
    for l in range(N_COEF):
        V = ss * cp[:, :, l]
        for s in range(5):
            j = l + s
            coef = KAPPA * BINOM[s]
            t = KNOTS[j]
            if j <= 3:
                for p, c in zip(range(4), (-t ** 3, 3 * t * t, -3 * t, 1.0)):
                    Wx[p] += c * coef * V
            elif j <= 10:
                Wr[j] += coef * V
    for j in J_FLIP:                 # relu(x-t)^3 = (x-t)^3 + relu(t-x)^3
        t = KNOTS[j]
        V = Wr[j]
        for p, c in zip(range(4), (-t ** 3, 3 * t * t, -3 * t, 1.0)):
            Wx[p] += c * V
    return Wx, Wr, sb


def _build_weights(control_points, scale_base, scale_spline, mask):
    """fp16 blob [128, 23*256], partition-major, chunks in PLANE_ORDER."""
    Wx, Wr, sb = _build_tables(control_points, scale_base, scale_spline, mask)
    ones_c = np.tile(Wx[0].sum(axis=0, keepdims=True) / 128.0, (128, 1))
    planes = {"x": Wx[1], "x2": Wx[2], "sil": sb, "x3": Wx[3]}
    for j in J_ALL:
        planes[f"z{j}"] = Wr[j]
    chunks = [ones_c.astype(np.float16)]
    for key in PLANE_ORDER[1:]:
        chunks += [planes[key][0:128].astype(np.float16),
                   planes[key][128:256].astype(np.float16)]
    W = np.stack(chunks, 0)                  # [23, 128, 256] f16
    w16 = np.ascontiguousarray(
        W.transpose(1, 0, 2).reshape(128, W_COLS))
    return w16


_DVE_LOCK = threading.Lock()
_DVE_OPS = {}


def _register_dve_ops():
    with _DVE_LOCK:
        if _DVE_OPS:
            return _DVE_OPS
        from concourse import dve_ops
        from concourse.dve_spec import Spec, Src0, C0, relu, sq, lower
        from concourse.dve_uop import DveOpSpec

        def mk(name, body, ref):
            for existing in dve_ops.OPS:
                if existing.name == name:
                    return existing
            spec = Spec(body=body, reference=ref)
            shas = {}
            for ver in ("v3", "v4"):
                tmp = DveOpSpec(name=name, opcode=0,
                                uops=lower(spec, ver=ver), rd1_en=False)
                shas[ver] = tmp.sha(ver)
            op = dve_ops.DveOp(name, spec, subdim=False, uops_sha=shas)
            dve_ops.OPS.append(op)
            dve_ops._SUB_OPCODE_FOR_NAME[name] = (
                dve_ops._CUSTOM_DVE_ROW_BASE + len(dve_ops.OPS) - 1)
            dve_ops.CUSTOM_DVE_SPECS[name] = spec
            return op

        rs = relu(Src0 + C0)
        _DVE_OPS["RELU3S"] = mk(
            "RELU3S_KAN_ANT", sq(rs) * rs,
            lambda in0, in1, s0, s1, imm2:
                np.maximum(in0.astype(np.float32) + s0, 0.0) ** 3)
        rf = relu(C0 - Src0)
        _DVE_OPS["RELU3F"] = mk(
            "RELU3F_KAN_ANT", sq(rf) * rf,
            lambda in0, in1, s0, s1, imm2:
                np.maximum(s0 - in0.astype(np.float32), 0.0) ** 3)
        _DVE_OPS["CUBE3"] = mk(
            "CUBE3_KAN_ANT", sq(Src0) * Src0,
            lambda in0, in1, s0, s1, imm2: in0.astype(np.float32) ** 3)
        return _DVE_OPS


_NC_LOCK = threading.Lock()
_NC_CACHE = {}


def _trace_bass():
    import concourse.mybir as mybir
    import concourse.tile as tile
    from concourse import bacc

    ops = _register_dve_ops()
    f32 = mybir.dt.float32
    f16 = mybir.dt.float16
    AFT = mybir.ActivationFunctionType

    nc = bacc.Bacc()
    xe = nc.dram_tensor("xe", [128, 512], f16, kind="ExternalInput")
    w16 = nc.dram_tensor("w16", [128, W_COLS], f16, kind="ExternalInput")
    out = nc.dram_tensor("out", [B_SHARD, OUT], f32, kind="ExternalOutput")

    with tile.TileContext(nc) as tc:
        with tc.tile_pool(name="p", bufs=1) as pool, \
             tc.tile_pool(name="ps", bufs=1, space="PSUM") as psum:
            # ---- DMA in: host-pretransposed fp16 xT first, then weights --
            xf = pool.tile([128, 512], f16, name="xf", tag="xf")
            nc.sync.dma_start(out=xf, in_=xe[:, :])
            wt = pool.tile([128, N_CHUNKS, 256], f16, name="wt", tag="wt")
            for g in range(len(W_GROUPS) - 1):
                c0, c1 = W_GROUPS[g], W_GROUPS[g + 1]
                nc.sync.dma_start(
                    out=wt[:, c0:c1, :],
                    in_=w16[:, c0 * 256:c1 * 256].rearrange(
                        "p (c o) -> p c o", o=256),
                )

            def wchunk(c):
                return wt[:, c, :]

            # ---- act-table warm (no DMA deps) ----
            dsrc = pool.tile([128, 1], f16, name="dsrc", tag="dsrc")
            nc.gpsimd.memset(dsrc, 1.0)
            dummy = pool.tile([128, 1], f16, name="dummy", tag="dummy")
            nc.scalar.activation(dummy, dsrc, AFT.Silu)

            # ---- features (fp16, [128,512] ops) ----
            feat = {"x": xf}
            feat["ones"] = pool.tile([128, 512], f16, name="onesf",
                                     tag="onesf")
            nc.gpsimd.memset(feat["ones"], 1.0)

            def ftile(name):
                return pool.tile([128, 512], f16, name=name, tag=name)

            # scalar: x2, silu
            feat["x2"] = ftile("x2f")
            nc.scalar.activation(feat["x2"], xf, AFT.Square)
            feat["sil"] = ftile("silf")
            nc.scalar.activation(feat["sil"], xf, AFT.Silu)
            # vector: custom fused planes
            def vec_plane(key, ap, in_ap):
                j = int(key[1:])
                t_j = float(KNOTS[j])
                if j in J_FLIP:
                    nc.vector._custom_dve(ops["RELU3F"], out=ap, in0=in_ap,
                                          s0=t_j)
                else:
                    nc.vector._custom_dve(ops["RELU3S"], out=ap, in0=in_ap,
                                          s0=-t_j)

            for key in VEC_PLANES:
                feat[key] = ftile(f"{key}f")
                vec_plane(key, feat[key], xf)
            # split planes: vector computes i-half 0, scalar+gpsimd half 1
            sg_half = {}
            for key in SPLIT_PLANES:
                j = int(key[1:])
                t_j = float(KNOTS[j])
                feat[key] = ftile(f"{key}f")
                vec_plane(key, feat[key][:, 0:256], xf[:, 0:256])
                r = pool.tile([128, 256], f16, name=f"rh{j}", tag=f"rh{j}")
                s = pool.tile([128, 256], f16, name=f"sh{j}", tag=f"sh{j}")
                bt = pool.tile([128, 1], f32, name=f"biash{j}",
                               tag=f"biash{j}")
                if j in J_FLIP:
                    nc.gpsimd.memset(bt, t_j)
                    nc.scalar.activation(r, xf[:, 256:512], AFT.Relu,
                                         bias=bt, scale=-1.0)
                else:
                    nc.gpsimd.memset(bt, -t_j)
                    nc.scalar.activation(r, xf[:, 256:512], AFT.Relu,
                                         bias=bt)
                nc.scalar.activation(s, r, AFT.Square)
                sg_half[key] = (r, s)
            for key in SPLIT_PLANES:
                r, s = sg_half[key]
                nc.gpsimd.tensor_mul(feat[key][:, 256:512], s, r)
            # scalar+gpsimd planes: r=relu(x-t) f16, s=r^2 f16, z=s*r f16
            for key in SG_PLANES:
                j = int(key[1:])
                t_j = float(KNOTS[j])
                r = pool.tile([128, 512], f16, name=f"r{j}", tag=f"r{j}")
                s = pool.tile([128, 512], f16, name=f"s{j}", tag=f"s{j}")
                if j in J_FLIP:
                    bt = pool.tile([128, 1], f32, name=f"bias{j}",
                                   tag=f"bias{j}")
                    nc.gpsimd.memset(bt, t_j)
                    nc.scalar.activation(r, xf, AFT.Relu, bias=bt, scale=-1.0)
                else:
                    bt = pool.tile([128, 1], f32, name=f"bias{j}",
                                   tag=f"bias{j}")
                    nc.gpsimd.memset(bt, -t_j)
                    nc.scalar.activation(r, xf, AFT.Relu, bias=bt)
                nc.scalar.activation(s, r, AFT.Square)
                feat[key] = ftile(f"{key}f")
                nc.gpsimd.tensor_mul(feat[key], s, r)
            # gpsimd: x^3 = x2 * x (f16)
            feat["x3"] = ftile("x3f")
            nc.gpsimd.tensor_mul(feat["x3"], feat["x2"], xf)

            # ---- matmuls: interleaved over 2 PSUM banks ----
            def chunk_feat(ci):
                if ci == 0:
                    return feat["ones"][:, 0:256]    # all-ones, any slice
                key = PLANE_ORDER[(ci + 1) // 2]
                h = (ci - 1) % 2
                return feat[key][:, h * 256:(h + 1) * 256]

            po = [psum.tile([128, 256], f32, name=f"po{bb}", tag=f"po{bb}")
                  for bb in range(2)]
            LAG = 2          # bank0 leads bank1 so its output drains early

            def mm(ci, bb):
                nc.tensor.matmul(
                    po[bb],
                    chunk_feat(ci)[:, bb * 128:(bb + 1) * 128],
                    wchunk(ci),
                    start=(ci == 0),
                    stop=(ci == N_CHUNKS - 1),
                )

            for i in range(N_CHUNKS + LAG):
                if i < N_CHUNKS:
                    mm(i, 0)
                if i >= LAG:
                    mm(i - LAG, 1)
            ob0 = pool.tile([128, 256], f32, name="ob0", tag="ob0")
            nc.scalar.copy(ob0, po[0])
            nc.sync.dma_start(out=out[0:128, :], in_=ob0)
            ob1 = pool.tile([128, 256], f32, name="ob1", tag="ob1")
            nc.vector.tensor_copy(ob1, po[1])
            nc.scalar.dma_start(out=out[128:256, :], in_=ob1)
    nc.finalize()
    return nc


def _get_nc():
    with _NC_LOCK:
        if "nc" not in _NC_CACHE:
            _NC_CACHE["nc"] = _trace_bass()
        return _NC_CACHE["nc"]


def kernel(x, knots, control_points, scale_base, scale_spline, mask):
    from concourse.bass_utils import run_bass_kernel_spmd

    x = np.ascontiguousarray(np.asarray(x, np.float32))
    W16 = _build_weights(control_points, scale_base, scale_spline, mask)
    nc = _get_nc()
    in_maps = []
    for c in range(N_CORES):
        xT = x[c * B_SHARD:(c + 1) * B_SHARD].astype(np.float16).T
        xe = np.ascontiguousarray(
            np.concatenate([xT[0:128], xT[128:256]], axis=1))
        in_maps.append({"xe": xe, "w16": W16})
    res = run_bass_kernel_spmd(
        nc, in_maps, core_ids=list(range(N_CORES)),
        trace=bool(int(os.environ.get("KAN_TRACE", "0"))),
    )
    out = np.concatenate([res.results[c]["out"] for c in range(N_CORES)],
                         axis=0)
    if res.exec_time_ns is not None:
        print(f"HW exec time: {res.exec_time_ns} ns")
    return out.astype(np.float32)
